# revision 4
# baseline (speedup 1.0000x reference)
"""Trainium2 Bass kernel for BinConv2d:
   y = relu(conv2d(sign(batchnorm_train(x)), W, pad=1) + b)

Sharding: data-parallel over batch, 4 images per core on 8 cores.
BN statistics are computed per-core (bn_stats/bn_aggr) and combined with a
tiny [128,2] AllReduce; sign() only depends on a per-channel threshold
(sign(gamma*(x-mean) + beta*sigma)), so the variance path never touches the
per-element math unless beta != 0.

Conv is 9 "taps" of a 64->64 matmul over all pixels. Binarized activations
(exact +-1 in fp16) are stored zero-padded [64ch, 114*114] per image, plus a
row-shifted duplicate on partitions 64..127 so that taps (kh,kw) and
(kh+1,kw) pair into one K=128 matmul. Two 4-row output chunks run
concurrently on the two column halves of the PE array via tile_position.
"""

import sys
from contextlib import ExitStack

import numpy as np

try:
    import concourse.bass as bass  # noqa: F401
except ImportError:  # pragma: no cover
    sys.path.insert(0, "/opt/trn_rl_repo")

import concourse.bacc as bacc
import concourse.tile as tile
from concourse import mybir
from concourse.bass_utils import run_bass_kernel_spmd

F32 = mybir.dt.float32
WDT = mybir.dt.float16  # dtype for conv weights and binarized activations

N_CORES = 8
N_IMG = 4  # images per core (batch 32 / 8 cores)
C = 64
H = 112
W = 112
HP = H + 2  # 114
WP = W + 2  # 114
IMG = HP * WP  # 12996
EPS = 1e-4

ROWS_PER_CHUNK = 4  # output rows per matmul chunk (N = 4*112 = 448)
NMM = ROWS_PER_CHUNK * W  # 448


def build_program(n_cores=N_CORES, n_img=N_IMG):
    """Builds the per-core Bass program (same program on every core)."""
    assert n_img % 2 == 0 or n_img == 1
    nhalf = max(n_img // 2, 1)
    fpart = nhalf * H * W  # free elems per partition of resident x
    n_halves = 2 if n_img >= 2 else 1
    total_count = n_cores * n_img * H * W  # global BN count per channel

    nc = bacc.Bacc(
        "TRN2", target_bir_lowering=False, debug=False, num_devices=n_cores
    )

    x = nc.dram_tensor("x", [n_img, C, H, W], F32, kind="ExternalInput")
    gamma = nc.dram_tensor("gamma", [C], F32, kind="ExternalInput")
    beta = nc.dram_tensor("beta", [C], F32, kind="ExternalInput")
    Wt = nc.dram_tensor("W", [C, C, 3, 3], F32, kind="ExternalInput")
    bt = nc.dram_tensor("b", [C], F32, kind="ExternalInput")
    y = nc.dram_tensor("y", [n_img, C, H, W], F32, kind="ExternalOutput")

    with tile.TileContext(nc) as tc, ExitStack() as ctx:
        const = ctx.enter_context(tc.tile_pool(name="const", bufs=1))
        bigp = ctx.enter_context(tc.tile_pool(name="big", bufs=1))
        xbp = ctx.enter_context(tc.tile_pool(name="xb", bufs=2))
        statp = ctx.enter_context(tc.tile_pool(name="stat", bufs=1))
        psump = ctx.enter_context(tc.tile_pool(name="ps", bufs=4, space="PSUM"))
        outp = ctx.enter_context(tc.tile_pool(name="out", bufs=4))
        dramp = ctx.enter_context(tc.tile_pool(name="dram", bufs=1, space="DRAM"))

        # ---------------- weights / constants prep ----------------
        # wstage[c, kh, kw, o] staging (f32) then cast to fp16.
        wstage = const.tile([C, 3, 3, C], F32)
        w_src = Wt.ap().rearrange("o c kh kw -> c kh kw o")
        for kh in range(3):
            for kw in range(3):
                nc.sync.dma_start(
                    out=wstage[:, kh, kw, :], in_=w_src[:, kh, kw, :]
                )
        # w2[0:64, t, o]  = tap t          (t = kh*3+kw)
        # w2[64:128, t, o] = tap t+3 (kh+1) for t in 0..5
        w2 = const.tile([128, 9, C], WDT)
        wsv = wstage.rearrange("c kh kw o -> c (kh kw) o")
        nc.vector.tensor_copy(out=w2[0:C, :, :], in_=wsv)
        nc.vector.tensor_copy(out=w2[C:128, 0:6, :], in_=w2[0:C, 3:9, :])

        b2 = const.tile([128, 1], F32)
        nc.sync.dma_start(out=b2[0:C, :], in_=bt.ap().rearrange("(c u) -> c u", u=1))
        nc.sync.dma_start(out=b2[C:128, :], in_=bt.ap().rearrange("(c u) -> c u", u=1))
        gamma2 = const.tile([128, 1], F32)
        nc.sync.dma_start(
            out=gamma2[0:C, :], in_=gamma.ap().rearrange("(c u) -> c u", u=1)
        )
        nc.sync.dma_start(
            out=gamma2[C:128, :], in_=gamma.ap().rearrange("(c u) -> c u", u=1)
        )
        beta64 = const.tile([C, 1], F32)
        nc.sync.dma_start(
            out=beta64, in_=beta.ap().rearrange("(c u) -> c u", u=1)
        )
        eps64 = const.tile([C, 1], F32)
        nc.vector.memset(eps64, EPS)

        # ---------------- load x + local BN stats ----------------
        # resident x: partition p = 64*half + c ; free = n2*12544 + h*112 + w
        xsb = bigp.tile([128, fpart], F32)
        xsb_v = xsb.rearrange("p (n2 h w) -> p n2 h w", n2=nhalf, h=H)

        n_bn = fpart // NMM  # bn_stats chunks of 448
        stats = statp.tile([128, n_bn, 6], F32)
        q_rows = 28  # DMA chunk rows
        n_q = H // q_rows
        bn_per_q = q_rows * W // NMM  # 7
        for n2 in range(nhalf):
            for q in range(n_q):
                for half in range(n_halves):
                    n = half * nhalf + n2
                    base = n2 * (H * W) + q * (q_rows * W)
                    dst = xsb[
                        half * C : half * C + C, base : base + q_rows * W
                    ].rearrange("c (h w) -> c h w", w=W)
                    nc.sync.dma_start(
                        out=dst,
                        in_=x.ap()[n, :, q * q_rows : (q + 1) * q_rows, :],
                    )
                for j in range(bn_per_q):
                    idx = (n2 * n_q + q) * bn_per_q + j
                    base = n2 * (H * W) + q * (q_rows * W) + j * NMM
                    nc.vector.bn_stats(
                        out=stats[:, idx, :], in_=xsb[:, base : base + NMM]
                    )

        mv = statp.tile([128, 2], F32)
        nc.vector.bn_aggr(out=mv, in_=stats)

        # ar payload: col0 = mean_p, col1 = E[x^2]_p = var_p + mean_p^2
        arin = statp.tile([128, 2], F32)
        nc.vector.tensor_copy(out=arin[:, 0:1], in_=mv[:, 0:1])
        msq = statp.tile([128, 1], F32)
        nc.vector.tensor_mul(out=msq, in0=mv[:, 0:1], in1=mv[:, 0:1])
        nc.vector.tensor_add(out=arin[:, 1:2], in0=mv[:, 1:2], in1=msq)

        cc_in = dramp.tile([128, 2], F32)
        cc_out = dramp.tile([128, 2], F32)
        nc.sync.dma_start(out=cc_in, in_=arin)
        if n_cores > 1:
            nc.gpsimd.collective_compute(
                "AllReduce",
                mybir.AluOpType.add,
                replica_groups=[list(range(n_cores))],
                ins=[cc_in[:].opt()],
                outs=[cc_out[:].opt()],
            )
        else:
            nc.gpsimd.dma_start(out=cc_out, in_=cc_in)
        ar = statp.tile([128, 2], F32)
        nc.sync.dma_start(out=ar, in_=cc_out)

        # fold partition halves -> per-channel global stats
        n_groups = n_cores * n_halves
        tot = statp.tile([C, 2], F32)
        if n_halves == 2:
            hi = statp.tile([C, 2], F32)
            nc.scalar.activation(
                out=hi, in_=ar[C:128, :], func=mybir.ActivationFunctionType.Copy
            )
            nc.vector.tensor_add(out=tot, in0=ar[0:C, :], in1=hi)
        else:
            nc.vector.tensor_copy(out=tot, in_=ar[0:C, :])
        mean64 = statp.tile([C, 1], F32)
        nc.vector.tensor_scalar_mul(mean64, tot[:, 0:1], 1.0 / n_groups)
        e2 = statp.tile([C, 1], F32)
        nc.vector.tensor_scalar_mul(e2, tot[:, 1:2], 1.0 / n_groups)
        var64 = statp.tile([C, 1], F32)
        nc.vector.tensor_mul(out=var64, in0=mean64, in1=mean64)
        nc.vector.tensor_sub(out=var64, in0=e2, in1=var64)
        sigma = statp.tile([C, 1], F32)
        nc.scalar.activation(
            out=sigma,
            in_=var64,
            func=mybir.ActivationFunctionType.Sqrt,
            bias=eps64,
        )
        # d = beta*sigma - gamma*mean ; binarize: xb = sign(gamma*x + d)
        d64 = statp.tile([C, 1], F32)
        nc.vector.tensor_mul(out=d64, in0=beta64, in1=sigma)
        t2 = statp.tile([C, 1], F32)
        nc.vector.tensor_mul(out=t2, in0=gamma2[0:C, :], in1=mean64)
        nc.vector.tensor_sub(out=d64, in0=d64, in1=t2)
        d2 = statp.tile([128, 1], F32)
        nc.vector.tensor_copy(out=d2[0:C, :], in_=d64)
        nc.scalar.activation(
            out=d2[C:128, :], in_=d64, func=mybir.ActivationFunctionType.Copy
        )

        # ---------------- per image: binarize + conv ----------------
        for n in range(n_img):
            half = n // nhalf
            n2 = n % nhalf
            xbt = xbp.tile([128, IMG], WDT, tag="xb")
            xbv = xbt.rearrange("p (hp wp) -> p hp wp", wp=WP)
            # zero borders of copy A (copy B inherits them)
            nc.vector.memset(xbv[0:C, 0:1, :], 0.0)
            nc.vector.memset(xbv[0:C, HP - 1 : HP, :], 0.0)
            nc.vector.memset(xbv[0:C, 1 : HP - 1, 0:1], 0.0)
            nc.vector.memset(xbv[0:C, 1 : HP - 1, WP - 1 : WP], 0.0)
            # binarize interior: xb = Sign(gamma * x + d)
            nc.scalar.activation(
                out=xbv[0:C, 1 : HP - 1, 1 : WP - 1],
                in_=xsb_v[half * C : half * C + C, n2, :, :],
                func=mybir.ActivationFunctionType.Sign,
                scale=gamma2[half * C : half * C + C, :],
                bias=d2[half * C : half * C + C, :],
            )
            # copy B: partitions 64..127 = copy A shifted by one padded row
            nc.vector.tensor_copy(
                out=xbt[C:128, 0 : IMG - WP], in_=xbt[0:C, WP:IMG]
            )

            n_slots = H // (2 * ROWS_PER_CHUNK)  # 14
            for s in range(n_slots):
                h0 = s * 2 * ROWS_PER_CHUNK
                h1 = h0 + ROWS_PER_CHUNK
                P = psump.tile([128, NMM], F32, tag="psum")
                mms = []
                # pairs (kh=0&1) then solos (kh=2); col groups interleaved
                for kw in range(3):
                    for cg, hb in ((0, h0), (64, h1)):
                        mms.append((cg, hb, kw, True))
                for kw in range(3):
                    for cg, hb in ((0, h0), (64, h1)):
                        mms.append((cg, hb, kw, False))
                # per-col-group start/stop: each col group clears and closes
                # its own partition slice of the PSUM bank
                cg_seen = set()
                cg_last = {cg: max(i for i, m in enumerate(mms) if m[0] == cg)
                           for cg in (0, 64)}
                for i, (cg, hb, kw, is_pair) in enumerate(mms):
                    if is_pair:
                        lhsT = w2[:, kw, :]
                        rhs = xbv[:, hb : hb + ROWS_PER_CHUNK, kw : kw + W]
                    else:
                        lhsT = w2[0:C, 6 + kw, :]
                        rhs = xbv[
                            0:C, hb + 2 : hb + 2 + ROWS_PER_CHUNK, kw : kw + W
                        ]
                    nc.tensor.matmul(
                        P[cg : cg + C, :],
                        lhsT,
                        rhs,
                        start=(cg not in cg_seen),
                        stop=(i == cg_last[cg]),
                        tile_position=(0, cg),
                        skip_group_check=True,
                    )
                    cg_seen.add(cg)
                # epilogue: relu(P + b)
                osb = outp.tile([128, NMM], F32, tag="osb")
                nc.scalar.activation(
                    out=osb,
                    in_=P,
                    func=mybir.ActivationFunctionType.Relu,
                    bias=b2,
                )
                ov = osb.rearrange("p (h w) -> p h w", w=W)
                nc.sync.dma_start(
                    out=y.ap()[n, :, h0 : h0 + ROWS_PER_CHUNK, :],
                    in_=ov[0:C, :, :],
                )
                nc.sync.dma_start(
                    out=y.ap()[n, :, h1 : h1 + ROWS_PER_CHUNK, :],
                    in_=ov[C:128, :, :],
                )

    nc.compile()
    return nc


_CACHE = {}


def _get_program(n_cores=N_CORES, n_img=N_IMG):
    key = (n_cores, n_img)
    if key not in _CACHE:
        _CACHE[key] = build_program(n_cores, n_img)
    return _CACHE[key]


def kernel(x, gamma, beta, W, b, _trace=False):
    x = np.ascontiguousarray(x, dtype=np.float32)
    n_total = x.shape[0]
    assert n_total == N_CORES * N_IMG, x.shape
    nc = _get_program(N_CORES, N_IMG)
    in_maps = []
    for c in range(N_CORES):
        in_maps.append(
            {
                "x": x[c * N_IMG : (c + 1) * N_IMG],
                "gamma": np.ascontiguousarray(gamma, np.float32),
                "beta": np.ascontiguousarray(beta, np.float32),
                "W": np.ascontiguousarray(W, np.float32),
                "b": np.ascontiguousarray(b, np.float32),
            }
        )
    res = run_bass_kernel_spmd(
        nc, in_maps, core_ids=list(range(N_CORES)), trace=_trace
    )
    out = np.concatenate([res.results[c]["y"] for c in range(N_CORES)], axis=0)
    if _trace:
        kernel._last_result = res
    return out
